# revision 19
# baseline (speedup 1.0000x reference)
"""BANLayer (bilinear attention network) Trainium2 kernel, 8-core data-parallel.

Strategy (per core, 4 of the 32 batches):
  - All big matmuls in fp32r (TF32: exact on host-pre-rounded inputs, bf16-rate).
  - k-dimension globally permuted (host side) so AvgPool groups of 3 become
    column-tile adds: new k position p maps to original k = 3*(p%512) + p//512.
  - q_kT [k,q] and v_kT [k,v] (k-major) feed the per-head bilinear matmul;
    v_n [v,k] feeds the fusion einsum contraction over v; the fusion
    reduction over q happens elementwise in k-major layout (no transposes).
  - Softmax over the flat (v,q) grid per (b,h): per-row shift + exp (ACT,
    fused row sums), global max/sum via gpsimd partition_all_reduce on
    [128,8] stat tiles (8 heads batched), then per-row rescale on ACT.
  - Software pipelining: projections of batch b+1 are emitted between the
    head loop and the softmax-stats tail of batch b so the PE never idles.
  - BatchNorm batch stats via an 8-core AllReduce of [128,8] partial sums.

kernel(**inputs) takes the FULL unsharded inputs and returns (logits, att)
exactly like the reference module.
"""

import os
import sys

import numpy as np

try:
    import concourse.bass as bass
except ImportError:  # fresh grading dir: fall back to the repo checkout
    sys.path.insert(0, "/opt/trn_rl_repo")
    import concourse.bass as bass

import concourse.bacc as bacc
import concourse.bass_isa as bass_isa
import concourse.mybir as mybir
import concourse.tile as tile
from concourse.bass_utils import run_bass_kernel_spmd

F32 = mybir.dt.float32
F32R = mybir.dt.float32r
BF16 = mybir.dt.bfloat16
AX = mybir.AxisListType
AF = mybir.ActivationFunctionType
ALU = mybir.AluOpType

B, NV, NQ = 32, 256, 512
V_DIM, Q_DIM, H_DIM, H_OUT, K = 128, 768, 512, 8, 3
KD = H_DIM * K  # 1536
N_CORES = 8
BL = int(os.environ.get("BAN_BL", B // N_CORES))  # local batches per core
KC = KD // 128  # 12 k-chunks
DC = Q_DIM // 128  # 6 d-chunks for q
NVT = NV // 128  # 2 v partition tiles
JC = H_DIM // 128  # 4 output feature chunks


def _tf32(x):
    b = np.ascontiguousarray(x, np.float32).view(np.uint32)
    b = ((b.astype(np.uint64) + 0x1000) & 0xFFFFE000).astype(np.uint32)
    return b.view(np.float32)


# new k position p holds original k index 3*(p % 512) + p // 512
_KPERM = (3 * (np.arange(KD) % H_DIM) + np.arange(KD) // H_DIM).astype(np.int64)


def _build(do_softmax, use_vmask, use_qmask, vb_zero, qb_zero):
    nc = bacc.Bacc(
        "TRN2", target_bir_lowering=False, debug=False, num_devices=N_CORES
    )

    # ---- DRAM I/O ----
    vT = nc.dram_tensor("vT", [BL, 128, NV], F32R, kind="ExternalInput")
    qT = nc.dram_tensor("qT", [BL, Q_DIM, NQ], F32R, kind="ExternalInput")
    qWT = nc.dram_tensor("qWT", [Q_DIM, KD], F32R, kind="ExternalInput")
    vWT = nc.dram_tensor("vWT", [128, KD], F32R, kind="ExternalInput")
    onesr = nc.dram_tensor("onesr", [1, 128], F32R, kind="ExternalInput")
    hmt = nc.dram_tensor("hmt", [128, KC * H_OUT], F32, kind="ExternalInput")
    bng = nc.dram_tensor("bng", [128, JC], F32, kind="ExternalInput")
    bnb = nc.dram_tensor("bnb", [128, JC], F32, kind="ExternalInput")
    hbrow = nc.dram_tensor("hbrow", [1, H_OUT], F32R, kind="ExternalInput")
    if not vb_zero:
        vbrow = nc.dram_tensor("vbrow", [1, KD], F32R, kind="ExternalInput")
        vbk = nc.dram_tensor("vbk", [128, KC], F32, kind="ExternalInput")
    if not qb_zero:
        qbk = nc.dram_tensor("qbk", [128, KC], F32, kind="ExternalInput")
    if use_vmask:
        vmadd = nc.dram_tensor("vmadd", [BL, NV], F32, kind="ExternalInput")
    if use_qmask:
        qmadd = nc.dram_tensor("qmadd", [BL, NQ], F32, kind="ExternalInput")

    att_o = nc.dram_tensor("att", [BL, H_OUT, NV, NQ], F32, kind="ExternalOutput")
    log_o = nc.dram_tensor("logits", [BL, H_DIM], F32, kind="ExternalOutput")

    with tile.TileContext(nc) as tc:
        with (
            tc.tile_pool(name="wts", bufs=1) as wts,
            tc.tile_pool(name="io", bufs=1) as io,
            tc.tile_pool(name="sb", bufs=1) as sb,
            tc.tile_pool(name="ps", bufs=1, space="PSUM") as ps,
            tc.tile_pool(name="dram", bufs=1, space="DRAM") as dram,
        ):
            # ---- load constants ----
            w_qwt = []
            for dc in range(DC):
                t = wts.tile([128, KD], F32R, name=f"w_qwt{dc}", tag="w_qwt", bufs=DC)
                nc.gpsimd.dma_start(out=t[:], in_=qWT[dc * 128 : (dc + 1) * 128, :])
                w_qwt.append(t)
            w_vwt = wts.tile([128, KD], F32R)
            nc.gpsimd.dma_start(out=w_vwt[:], in_=vWT[:])
            w_ones = wts.tile([1, 128], F32R)
            nc.sync.dma_start(out=w_ones[:], in_=onesr[:])
            w_hmt = wts.tile([128, KC * H_OUT], F32)
            nc.sync.dma_start(out=w_hmt[:], in_=hmt[:])
            w_bng = wts.tile([128, JC], F32)
            nc.sync.dma_start(out=w_bng[:], in_=bng[:])
            w_bnb = wts.tile([128, JC], F32)
            nc.sync.dma_start(out=w_bnb[:], in_=bnb[:])
            w_hbrow = wts.tile([1, H_OUT], F32R)
            nc.sync.dma_start(out=w_hbrow[:], in_=hbrow[:])
            if not vb_zero:
                w_vbrow = wts.tile([1, KD], F32R)
                nc.sync.dma_start(out=w_vbrow[:], in_=vbrow[:])
                w_vbk = wts.tile([128, KC], F32)
                nc.sync.dma_start(out=w_vbk[:], in_=vbk[:])
            if not qb_zero:
                w_qbk = wts.tile([128, KC], F32)
                nc.sync.dma_start(out=w_qbk[:], in_=qbk[:])

            w_eps = wts.tile([128, 1], F32)
            nc.vector.memset(w_eps[:], 1e-10)

            hb_cols = None
            if not do_softmax:
                ps_hb = ps.tile([128, H_OUT], F32, tag="ps_proj", bufs=2)
                nc.tensor.matmul(
                    ps_hb[:], w_ones[:], w_hbrow[:], start=True, stop=True
                )
                hb_cols = sb.tile([128, H_OUT], F32)
                nc.vector.tensor_copy(hb_cols[:], ps_hb[:])

            # P[p, jc, b] accumulates pooled features
            P = sb.tile([128, JC, BL], F32)
            LG = sb.tile([128, JC, BL], F32)

            # -------- emission helpers (each emits IR; Tile schedules) ------
            loads = {}

            def emit_loads(b):
                d = {}
                t = io.tile([128, NV], F32R, name=f"vt_in{b}", tag="vt", bufs=2)
                nc.sync.dma_start(out=t[:], in_=vT[b])
                d["vt_in"] = t
                d["qt_in"] = []
                for dc in range(DC):
                    t = io.tile(
                        [128, NQ], F32R, name=f"qt_in{b}_{dc}", tag="qt", bufs=10
                    )
                    nc.sync.dma_start(
                        out=t[:], in_=qT[b, dc * 128 : (dc + 1) * 128, :]
                    )
                    d["qt_in"].append(t)
                if use_vmask:
                    t = io.tile([128, NVT], F32, name=f"vm{b}", tag="vm", bufs=2)
                    nc.sync.dma_start(
                        out=t[:], in_=vmadd[b].rearrange("(vt p) -> p vt", p=128)
                    )
                    d["vm_c"] = t
                if use_qmask:
                    qm_r = io.tile([1, NQ], F32, name=f"qmr{b}", tag="qmr", bufs=2)
                    nc.sync.dma_start(out=qm_r[:], in_=qmadd[b : b + 1, :])
                    qm_f = io.tile([128, NQ], F32, name=f"qmf{b}", tag="qmf", bufs=2)
                    nc.gpsimd.partition_broadcast(qm_f[:], qm_r[:], channels=128)
                    d["qm_f"] = qm_f
                loads[b] = d

            projs = {}

            def emit_proj(b):
                d = loads[b]
                vt_in, qt_in = d["vt_in"], d["qt_in"]
                # q_kT: [k,q] k-major, relu(+bias)
                qk = []
                for kc in range(KC):
                    psq = ps.tile(
                        [128, NQ], F32, name=f"psq{b}_{kc}", tag="ps_proj", bufs=2
                    )
                    for dc in range(DC):
                        nc.tensor.matmul(
                            psq[:],
                            w_qwt[dc][:, kc * 128 : (kc + 1) * 128],
                            qt_in[dc][:],
                            start=(dc == 0),
                            stop=(dc == DC - 1),
                        )
                    t = sb.tile([128, NQ], F32R, name=f"qk{b}_{kc}", tag="qk", bufs=24)
                    qb_bias = 0.0 if qb_zero else w_qbk[:, kc : kc + 1]
                    nc.scalar.activation(
                        t[:], psq[:], AF.Relu, bias=qb_bias, scale=1.0
                    )
                    qk.append(t)
                # v_n: [v,k] natural layout, relu (+bias via rank-1 matmul)
                vn = []
                for vt in range(NVT):
                    t = sb.tile([128, KD], F32R, name=f"vn{b}_{vt}", tag="vn", bufs=3)
                    for ks in range(KD // NQ):
                        psv = ps.tile(
                            [128, NQ], F32, name=f"psv{b}_{vt}_{ks}", tag="ps_proj",
                            bufs=2,
                        )
                        nc.tensor.matmul(
                            psv[:],
                            vt_in[:, vt * 128 : (vt + 1) * 128],
                            w_vwt[:, ks * NQ : (ks + 1) * NQ],
                            start=True,
                            stop=vb_zero,
                        )
                        if not vb_zero:
                            nc.tensor.matmul(
                                psv[:],
                                w_ones[:],
                                w_vbrow[:, ks * NQ : (ks + 1) * NQ],
                                start=False,
                                stop=True,
                            )
                        nc.scalar.activation(
                            t[:, ks * NQ : (ks + 1) * NQ], psv[:], AF.Relu
                        )
                    vn.append(t)
                # v_kT: [k,v] k-major, relu(+bias)
                vk = []
                for kc in range(KC):
                    psk = ps.tile(
                        [128, NV], F32, name=f"psk{b}_{kc}", tag="ps_proj", bufs=2
                    )
                    nc.tensor.matmul(
                        psk[:],
                        w_vwt[:, kc * 128 : (kc + 1) * 128],
                        vt_in[:],
                        start=True,
                        stop=True,
                    )
                    t = sb.tile([128, NV], F32R, name=f"vk{b}_{kc}", tag="vk", bufs=12)
                    vb_bias = 0.0 if vb_zero else w_vbk[:, kc : kc + 1]
                    nc.scalar.activation(
                        t[:], psk[:], AF.Relu, bias=vb_bias, scale=1.0
                    )
                    vk.append(t)
                projs[b] = (qk, vn, vk)

            def emit_heads(b, pending=None):
                qk, vn, vk = projs[b]
                d = loads[b]
                nm = [
                    sb.tile([128, H_OUT], F32, name=f"nm{b}_{vt}", tag="nm", bufs=4)
                    for vt in range(NVT)
                ]
                rr = [
                    sb.tile([128, H_OUT], F32, name=f"rr{b}_{vt}", tag="rr", bufs=4)
                    for vt in range(NVT)
                ]
                att_sum = [
                    sb.tile([128, NQ], F32R, name=f"asum{b}_{vt}", tag="asum", bufs=4)
                    for vt in range(NVT)
                ]
                zs = {}

                def emit_vh(h):
                    vhs = []
                    for kc in range(KC):
                        t = sb.tile(
                            [128, NV], F32R, name=f"vh{b}_{h}_{kc}", tag="vh", bufs=26
                        )
                        c = kc * H_OUT + h
                        nc.vector.tensor_scalar_mul(
                            t[:], vk[kc][:].bitcast(F32), w_hmt[:, c : c + 1]
                        )
                        vhs.append(t)
                    return vhs

                vh_next = emit_vh(0)
                for h in range(H_OUT):
                    vh = vh_next
                    if h + 1 < H_OUT:
                        vh_next = emit_vh(h + 1)
                    psa = [
                        ps.tile(
                            [128, NQ], F32, name=f"psa{b}_{h}_{vt}", tag="ps_att",
                            bufs=4,
                        )
                        for vt in range(NVT)
                    ]
                    for kc in range(KC):
                        for vt in range(NVT):
                            nc.tensor.matmul(
                                psa[vt][:],
                                vh[kc][:, vt * 128 : (vt + 1) * 128],
                                qk[kc][:],
                                start=(kc == 0),
                                stop=(kc == KC - 1),
                            )
                    for vt in range(NVT):
                        if do_softmax:
                            if use_vmask:
                                nc.vector.tensor_scalar_add(
                                    psa[vt][:], psa[vt][:], d["vm_c"][:, vt : vt + 1]
                                )
                            if use_qmask:
                                nc.vector.tensor_add(
                                    psa[vt][:], psa[vt][:], d["qm_f"][:]
                                )
                            nc.vector.reduce_max(
                                nm[vt][:, h : h + 1], psa[vt][:], axis=AX.X,
                                negate=True,
                            )
                            z = sb.tile(
                                [128, NQ], BF16, name=f"z{b}_{h}_{vt}", tag="z",
                                bufs=18,
                            )
                            nc.scalar.activation(
                                z[:],
                                psa[vt][:],
                                AF.Exp,
                                bias=nm[vt][:, h : h + 1],
                                scale=1.0,
                                accum_out=rr[vt][:, h : h + 1],
                            )
                            zs[(h, vt)] = z
                        else:
                            ao = sb.tile(
                                [128, NQ], F32, name=f"ao{b}_{h}_{vt}", tag="ao",
                                bufs=3,
                            )
                            nc.vector.tensor_scalar_add(
                                ao[:], psa[vt][:], hb_cols[:, h : h + 1]
                            )
                            nc.sync.dma_start(
                                out=att_o[b, h, vt * 128 : (vt + 1) * 128, :],
                                in_=ao[:],
                            )
                            if h == 0:
                                nc.vector.tensor_copy(att_sum[vt][:], ao[:])
                            else:
                                nc.vector.tensor_add(
                                    att_sum[vt][:],
                                    ao[:],
                                    att_sum[vt][:].bitcast(F32),
                                )
                    # interleave deferred normalize/DMA work from the previous batch
                    if pending:
                        for _ in range(2):
                            if pending:
                                pending.pop(0)()
                while pending:
                    pending.pop(0)()
                return nm, rr, att_sum, zs

            def emit_stats_accum(b, nm, rr, att_sum, zs):
                # global per-(b,h) softmax constants from per-row stats
                m = []
                for vt in range(NVT):
                    t = sb.tile([128, H_OUT], F32, name=f"m{b}_{vt}", tag="mm", bufs=4)
                    nc.vector.tensor_scalar_mul(t[:], nm[vt][:], -1.0)
                    m.append(t)
                mg = sb.tile([128, H_OUT], F32, name=f"mg{b}", tag="mg", bufs=4)
                nc.vector.tensor_max(mg[:], m[0][:], m[1][:])
                Mt = sb.tile([128, H_OUT], F32, name=f"Mt{b}", tag="Mt", bufs=4)
                nc.gpsimd.partition_all_reduce(
                    Mt[:], mg[:], channels=128, reduce_op=bass_isa.ReduceOp.max
                )
                ws = sb.tile([128, H_OUT], F32, name=f"ws{b}", tag="wsu", bufs=4)
                ee = []
                w0 = None
                for vt in range(NVT):
                    dd = sb.tile([128, H_OUT], F32, name=f"d{b}_{vt}", tag="dd", bufs=4)
                    nc.vector.tensor_sub(dd[:], m[vt][:], Mt[:])
                    e = sb.tile([128, H_OUT], F32, name=f"e{b}_{vt}", tag="ee", bufs=4)
                    nc.scalar.activation(e[:], dd[:], AF.Exp)
                    ee.append(e)
                    w = sb.tile([128, H_OUT], F32, name=f"wv{b}_{vt}", tag="wv", bufs=4)
                    nc.vector.tensor_mul(w[:], rr[vt][:], e[:])
                    if vt == 0:
                        w0 = w
                    else:
                        nc.vector.tensor_add(ws[:], w0[:], w[:])
                St = sb.tile([128, H_OUT], F32, name=f"St{b}", tag="Stt", bufs=4)
                nc.gpsimd.partition_all_reduce(
                    St[:], ws[:], channels=128, reduce_op=bass_isa.ReduceOp.add
                )
                rS = sb.tile([128, H_OUT], F32, name=f"rS{b}", tag="rSS", bufs=4)
                nc.vector.reciprocal(rS[:], St[:])
                cf = []
                for vt in range(NVT):
                    t = sb.tile(
                        [128, H_OUT], F32, name=f"cf{b}_{vt}", tag="cff", bufs=4
                    )
                    nc.vector.tensor_mul(t[:], ee[vt][:], rS[:])
                    cf.append(t)
                # accumulate att_sum straight from z so fusion can start ASAP
                for h in range(H_OUT):
                    for vt in range(NVT):
                        z = zs[(h, vt)]
                        if h == 0:
                            nc.vector.tensor_scalar(
                                att_sum[vt][:],
                                z[:],
                                cf[vt][:, h : h + 1],
                                float(H_OUT) * 1e-10,
                                op0=ALU.mult,
                                op1=ALU.add,
                            )
                        else:
                            nc.vector.scalar_tensor_tensor(
                                att_sum[vt][:],
                                z[:],
                                cf[vt][:, h : h + 1],
                                att_sum[vt][:].bitcast(F32),
                                op0=ALU.mult,
                                op1=ALU.add,
                            )
                # deferred per-head normalize (ACT) + DMA closures
                pending = []
                for h in range(H_OUT):
                    for vt in range(NVT):
                        def mk(h=h, vt=vt):
                            z = zs[(h, vt)]
                            ao = sb.tile(
                                [128, NQ], F32, name=f"ao{b}_{h}_{vt}", tag="ao",
                                bufs=3,
                            )
                            nc.scalar.activation(
                                ao[:],
                                z[:],
                                AF.Identity,
                                bias=w_eps[:],
                                scale=cf[vt][:, h : h + 1],
                            )
                            eng = nc.sync if vt == 0 else nc.gpsimd
                            eng.dma_start(
                                out=att_o[b, h, vt * 128 : (vt + 1) * 128, :],
                                in_=ao[:],
                            )
                        pending.append(mk)
                return pending

            def emit_fusion(b, att_sum):
                qk, vn, vk = projs[b]
                fu = sb.tile([128, KC], F32, name=f"fu{b}", tag="fu", bufs=2)
                for kc in range(KC):
                    psu = ps.tile(
                        [128, NQ], F32, name=f"psu{b}_{kc}", tag="ps_fus", bufs=2
                    )
                    for vt in range(NVT):
                        nc.tensor.matmul(
                            psu[:],
                            vn[vt][:, kc * 128 : (kc + 1) * 128],
                            att_sum[vt][:],
                            start=(vt == 0),
                            stop=(vt == NVT - 1),
                        )
                    fscr = sb.tile(
                        [128, NQ], F32, name=f"fscr{b}_{kc}", tag="fscr", bufs=2
                    )
                    nc.vector.scalar_tensor_tensor(
                        fscr[:],
                        psu[:],
                        1.0,
                        qk[kc][:].bitcast(F32),
                        op0=ALU.mult,
                        op1=ALU.mult,
                        accum_out=fu[:, kc : kc + 1],
                    )
                t4 = sb.tile([128, JC], F32, name=f"t4{b}", tag="st4", bufs=2)
                nc.vector.tensor_add(t4[:], fu[:, 0:JC], fu[:, JC : 2 * JC])
                nc.vector.tensor_add(P[:, :, b], t4[:], fu[:, 2 * JC : 3 * JC])
                del projs[b]

            # -------- software-pipelined emission --------
            pending = None
            emit_loads(0)
            for b in range(BL):
                if b == 0:
                    emit_proj(0)
                if b + 1 < BL:
                    emit_loads(b + 1)
                nm, rr, att_sum, zs = emit_heads(b, pending)
                if b + 1 < BL:
                    emit_proj(b + 1)
                if do_softmax:
                    pending = emit_stats_accum(b, nm, rr, att_sum, zs)
                emit_fusion(b, att_sum)
            while pending:
                pending.pop(0)()

            # ---- BatchNorm over the full batch via AllReduce ----
            S12 = sb.tile([128, 2 * JC], F32)
            nc.vector.reduce_sum(S12[:, 0:JC], P[:], axis=AX.X)
            Psq = sb.tile([128, JC, BL], F32)
            nc.vector.tensor_mul(Psq[:], P[:], P[:])
            nc.vector.reduce_sum(S12[:, JC : 2 * JC], Psq[:], axis=AX.X)

            ccin = dram.tile([128, 2 * JC], F32)
            ccout = nc.dram_tensor("ccout", [128, 2 * JC], F32, addr_space="Shared")
            nc.gpsimd.dma_start(out=ccin[:], in_=S12[:])
            nc.gpsimd.collective_compute(
                "AllReduce",
                ALU.add,
                replica_groups=[list(range(N_CORES))],
                ins=[ccin[:]],
                outs=[ccout[:]],
            )
            Rt = sb.tile([128, 2 * JC], F32)
            nc.gpsimd.dma_start(out=Rt[:], in_=ccout[:])

            mn = sb.tile([128, JC], F32)
            nc.vector.tensor_scalar_mul(mn[:], Rt[:, 0:JC], 1.0 / B)
            e2 = sb.tile([128, JC], F32)
            nc.vector.tensor_scalar_mul(e2[:], Rt[:, JC : 2 * JC], 1.0 / B)
            vr = sb.tile([128, JC], F32)
            nc.vector.tensor_mul(vr[:], mn[:], mn[:])
            nc.vector.tensor_sub(vr[:], e2[:], vr[:])
            nc.vector.tensor_scalar_add(vr[:], vr[:], 1e-5)
            sd = sb.tile([128, JC], F32)
            nc.scalar.activation(sd[:], vr[:], AF.Sqrt)
            rstd = sb.tile([128, JC], F32)
            nc.vector.reciprocal(rstd[:], sd[:])
            sc = sb.tile([128, JC], F32)
            nc.vector.tensor_mul(sc[:], w_bng[:], rstd[:])
            for b in range(BL):
                t1 = sb.tile([128, JC], F32, name=f"lg1{b}", tag="st4", bufs=2)
                nc.vector.tensor_sub(t1[:], P[:, :, b], mn[:])
                nc.vector.tensor_mul(t1[:], t1[:], sc[:])
                nc.vector.tensor_add(LG[:, :, b], t1[:], w_bnb[:])
            for b in range(BL):
                nc.gpsimd.dma_start(
                    out=log_o[b].rearrange("(jc p) -> p jc", p=128),
                    in_=LG[:, :, b],
                )

    nc.compile()
    return nc


_CACHE = {}
_LAST_IN_MAPS = None


def _get_nc(key):
    if key not in _CACHE:
        _CACHE[key] = _build(*key)
    return _CACHE[key]


def kernel(
    v, q, v_mask, q_mask, softmax, v_W, v_b, q_W, q_b, h_mat, h_bias,
    bn_gamma, bn_beta,
):
    v = np.asarray(v, np.float32)
    q = np.asarray(q, np.float32)
    v_mask = np.asarray(v_mask)
    q_mask = np.asarray(q_mask)
    do_softmax = bool(np.asarray(softmax).item())
    use_vmask = do_softmax and not bool(np.all(v_mask != 0))
    use_qmask = do_softmax and not bool(np.all(q_mask != 0))
    vb_zero = bool(np.all(np.asarray(v_b) == 0))
    qb_zero = bool(np.all(np.asarray(q_b) == 0))

    kp = _KPERM
    v_Wp = np.asarray(v_W, np.float32)[kp]
    q_Wp = np.asarray(q_W, np.float32)[kp]
    v_bp = np.asarray(v_b, np.float32)[kp]
    q_bp = np.asarray(q_b, np.float32)[kp]
    h_mp = np.asarray(h_mat, np.float32)[:, kp]

    nc = _get_nc((do_softmax, use_vmask, use_qmask, vb_zero, qb_zero))

    # host-side shared (replicated) tensors
    qWT = _tf32(np.ascontiguousarray(q_Wp.T))  # [768, 1536]
    vWT = _tf32(np.ascontiguousarray(v_Wp.T))  # [128, 1536]
    onesr = np.ones((1, 128), np.float32)
    # hmt[p, kc*8+h] = h_mp[h, kc*128+p]
    hmt = np.ascontiguousarray(
        h_mp.reshape(H_OUT, KC, 128).transpose(2, 1, 0).reshape(128, KC * H_OUT)
    ).astype(np.float32)
    bng = np.ascontiguousarray(
        np.asarray(bn_gamma, np.float32).reshape(JC, 128).T
    )
    bnb = np.ascontiguousarray(
        np.asarray(bn_beta, np.float32).reshape(JC, 128).T
    )
    hbrow = _tf32(np.asarray(h_bias, np.float32)[None, :])

    in_maps = []
    for c in range(N_CORES):
        sl = slice(c * BL, (c + 1) * BL)
        m = {
            "vT": _tf32(np.ascontiguousarray(v[sl].transpose(0, 2, 1))),
            "qT": _tf32(np.ascontiguousarray(q[sl].transpose(0, 2, 1))),
            "qWT": qWT,
            "vWT": vWT,
            "onesr": onesr,
            "hmt": hmt,
            "bng": bng,
            "bnb": bnb,
            "hbrow": hbrow,
        }
        if not vb_zero:
            m["vbrow"] = _tf32(v_bp[None, :])
            m["vbk"] = np.ascontiguousarray(v_bp.reshape(KC, 128).T).astype(
                np.float32
            )
        if not qb_zero:
            m["qbk"] = np.ascontiguousarray(q_bp.reshape(KC, 128).T).astype(
                np.float32
            )
        if use_vmask:
            m["vmadd"] = ((v_mask[sl] != 0).astype(np.float32) - 1.0) * 1e9
        if use_qmask:
            m["qmadd"] = ((q_mask[sl] != 0).astype(np.float32) - 1.0) * 1e9
        in_maps.append(m)

    global _LAST_IN_MAPS
    _LAST_IN_MAPS = in_maps
    res = run_bass_kernel_spmd(nc, in_maps, list(range(N_CORES)))

    att = np.concatenate([res.results[c]["att"] for c in range(N_CORES)], axis=0)
    logits = np.concatenate(
        [res.results[c]["logits"] for c in range(N_CORES)], axis=0
    )
    return logits, att


# revision 20
# speedup vs baseline: 1.0211x; 1.0211x over previous
"""BANLayer (bilinear attention network) Trainium2 kernel, 8-core data-parallel.

Strategy (per core, 4 of the 32 batches):
  - All big matmuls in fp32r (TF32: exact on host-pre-rounded inputs, bf16-rate).
  - k-dimension globally permuted (host side) so AvgPool groups of 3 become
    column-tile adds: new k position p maps to original k = 3*(p%512) + p//512.
  - q_kT [k,q] and v_kT [k,v] (k-major) feed the per-head bilinear matmul;
    v_n [v,k] feeds the fusion einsum contraction over v; the fusion
    reduction over q happens elementwise in k-major layout (no transposes).
  - Softmax over the flat (v,q) grid per (b,h): per-row shift + exp (ACT,
    fused row sums), global max/sum via gpsimd partition_all_reduce on
    [128,8] stat tiles (8 heads batched), then per-row rescale on ACT.
  - Software pipelining: projections of batch b+1 are emitted between the
    head loop and the softmax-stats tail of batch b so the PE never idles.
  - BatchNorm batch stats via an 8-core AllReduce of [128,8] partial sums.

kernel(**inputs) takes the FULL unsharded inputs and returns (logits, att)
exactly like the reference module.
"""

import os
import sys

import numpy as np

try:
    import concourse.bass as bass
except ImportError:  # fresh grading dir: fall back to the repo checkout
    sys.path.insert(0, "/opt/trn_rl_repo")
    import concourse.bass as bass

import concourse.bacc as bacc
import concourse.bass_isa as bass_isa
import concourse.mybir as mybir
import concourse.tile as tile
from concourse.bass_utils import run_bass_kernel_spmd

F32 = mybir.dt.float32
F32R = mybir.dt.float32r
BF16 = mybir.dt.bfloat16
AX = mybir.AxisListType
AF = mybir.ActivationFunctionType
ALU = mybir.AluOpType

B, NV, NQ = 32, 256, 512
V_DIM, Q_DIM, H_DIM, H_OUT, K = 128, 768, 512, 8, 3
KD = H_DIM * K  # 1536
N_CORES = 8
BL = int(os.environ.get("BAN_BL", B // N_CORES))  # local batches per core
KC = KD // 128  # 12 k-chunks
DC = Q_DIM // 128  # 6 d-chunks for q
NVT = NV // 128  # 2 v partition tiles
JC = H_DIM // 128  # 4 output feature chunks


def _tf32(x):
    b = np.ascontiguousarray(x, np.float32).view(np.uint32)
    b = ((b.astype(np.uint64) + 0x1000) & 0xFFFFE000).astype(np.uint32)
    return b.view(np.float32)


# new k position p holds original k index 3*(p % 512) + p // 512
_KPERM = (3 * (np.arange(KD) % H_DIM) + np.arange(KD) // H_DIM).astype(np.int64)


def _build(do_softmax, use_vmask, use_qmask, vb_zero, qb_zero):
    nc = bacc.Bacc(
        "TRN2", target_bir_lowering=False, debug=False, num_devices=N_CORES
    )

    # ---- DRAM I/O ----
    vT = nc.dram_tensor("vT", [BL, 128, NV], F32R, kind="ExternalInput")
    qT = nc.dram_tensor("qT", [BL, Q_DIM, NQ], F32R, kind="ExternalInput")
    qWT = nc.dram_tensor("qWT", [Q_DIM, KD], F32R, kind="ExternalInput")
    vWT = nc.dram_tensor("vWT", [128, KD], F32R, kind="ExternalInput")
    onesr = nc.dram_tensor("onesr", [1, 128], F32R, kind="ExternalInput")
    hmt = nc.dram_tensor("hmt", [128, KC * H_OUT], F32, kind="ExternalInput")
    bng = nc.dram_tensor("bng", [128, JC], F32, kind="ExternalInput")
    bnb = nc.dram_tensor("bnb", [128, JC], F32, kind="ExternalInput")
    hbrow = nc.dram_tensor("hbrow", [1, H_OUT], F32R, kind="ExternalInput")
    if not vb_zero:
        vbrow = nc.dram_tensor("vbrow", [1, KD], F32R, kind="ExternalInput")
        vbk = nc.dram_tensor("vbk", [128, KC], F32, kind="ExternalInput")
    if not qb_zero:
        qbk = nc.dram_tensor("qbk", [128, KC], F32, kind="ExternalInput")
    if use_vmask:
        vmadd = nc.dram_tensor("vmadd", [BL, NV], F32, kind="ExternalInput")
    if use_qmask:
        qmadd = nc.dram_tensor("qmadd", [BL, NQ], F32, kind="ExternalInput")

    att_o = nc.dram_tensor("att", [BL, H_OUT, NV, NQ], F32, kind="ExternalOutput")
    log_o = nc.dram_tensor("logits", [BL, H_DIM], F32, kind="ExternalOutput")

    with tile.TileContext(nc) as tc:
        with (
            tc.tile_pool(name="wts", bufs=1) as wts,
            tc.tile_pool(name="io", bufs=1) as io,
            tc.tile_pool(name="sb", bufs=1) as sb,
            tc.tile_pool(name="ps", bufs=1, space="PSUM") as ps,
            tc.tile_pool(name="dram", bufs=1, space="DRAM") as dram,
        ):
            # ---- load constants ----
            w_qwt = []
            for dc in range(DC):
                t = wts.tile([128, KD], F32R, name=f"w_qwt{dc}", tag="w_qwt", bufs=DC)
                nc.gpsimd.dma_start(out=t[:], in_=qWT[dc * 128 : (dc + 1) * 128, :])
                w_qwt.append(t)
            w_vwt = wts.tile([128, KD], F32R)
            nc.gpsimd.dma_start(out=w_vwt[:], in_=vWT[:])
            w_ones = wts.tile([1, 128], F32R)
            nc.sync.dma_start(out=w_ones[:], in_=onesr[:])
            w_hmt = wts.tile([128, KC * H_OUT], F32)
            nc.sync.dma_start(out=w_hmt[:], in_=hmt[:])
            w_bng = wts.tile([128, JC], F32)
            nc.sync.dma_start(out=w_bng[:], in_=bng[:])
            w_bnb = wts.tile([128, JC], F32)
            nc.sync.dma_start(out=w_bnb[:], in_=bnb[:])
            w_hbrow = wts.tile([1, H_OUT], F32R)
            nc.sync.dma_start(out=w_hbrow[:], in_=hbrow[:])
            if not vb_zero:
                w_vbrow = wts.tile([1, KD], F32R)
                nc.sync.dma_start(out=w_vbrow[:], in_=vbrow[:])
                w_vbk = wts.tile([128, KC], F32)
                nc.sync.dma_start(out=w_vbk[:], in_=vbk[:])
            if not qb_zero:
                w_qbk = wts.tile([128, KC], F32)
                nc.sync.dma_start(out=w_qbk[:], in_=qbk[:])

            w_eps = wts.tile([128, 1], F32)
            nc.vector.memset(w_eps[:], 1e-10)

            hb_cols = None
            if not do_softmax:
                ps_hb = ps.tile([128, H_OUT], F32, tag="ps_proj", bufs=2)
                nc.tensor.matmul(
                    ps_hb[:], w_ones[:], w_hbrow[:], start=True, stop=True
                )
                hb_cols = sb.tile([128, H_OUT], F32)
                nc.vector.tensor_copy(hb_cols[:], ps_hb[:])

            # P[p, jc, b] accumulates pooled features
            P = sb.tile([128, JC, BL], F32)
            LG = sb.tile([128, JC, BL], F32)

            # -------- emission helpers (each emits IR; Tile schedules) ------
            loads = {}

            def emit_loads(b):
                d = {}
                t = io.tile([128, NV], F32R, name=f"vt_in{b}", tag="vt", bufs=2)
                nc.sync.dma_start(out=t[:], in_=vT[b])
                d["vt_in"] = t
                d["qt_in"] = []
                for dc in range(DC):
                    t = io.tile(
                        [128, NQ], F32R, name=f"qt_in{b}_{dc}", tag="qt", bufs=10
                    )
                    nc.sync.dma_start(
                        out=t[:], in_=qT[b, dc * 128 : (dc + 1) * 128, :]
                    )
                    d["qt_in"].append(t)
                if use_vmask:
                    t = io.tile([128, NVT], F32, name=f"vm{b}", tag="vm", bufs=2)
                    nc.sync.dma_start(
                        out=t[:], in_=vmadd[b].rearrange("(vt p) -> p vt", p=128)
                    )
                    d["vm_c"] = t
                if use_qmask:
                    qm_r = io.tile([1, NQ], F32, name=f"qmr{b}", tag="qmr", bufs=2)
                    nc.sync.dma_start(out=qm_r[:], in_=qmadd[b : b + 1, :])
                    qm_f = io.tile([128, NQ], F32, name=f"qmf{b}", tag="qmf", bufs=2)
                    nc.gpsimd.partition_broadcast(qm_f[:], qm_r[:], channels=128)
                    d["qm_f"] = qm_f
                loads[b] = d

            projs = {}

            def emit_proj(b):
                d = loads[b]
                vt_in, qt_in = d["vt_in"], d["qt_in"]
                # q_kT: [k,q] k-major, relu(+bias)
                qk = []
                for kc in range(KC):
                    psq = ps.tile(
                        [128, NQ], F32, name=f"psq{b}_{kc}", tag="ps_proj", bufs=2
                    )
                    for dc in range(DC):
                        nc.tensor.matmul(
                            psq[:],
                            w_qwt[dc][:, kc * 128 : (kc + 1) * 128],
                            qt_in[dc][:],
                            start=(dc == 0),
                            stop=(dc == DC - 1),
                        )
                    t = sb.tile([128, NQ], F32R, name=f"qk{b}_{kc}", tag="qk", bufs=24)
                    qb_bias = 0.0 if qb_zero else w_qbk[:, kc : kc + 1]
                    nc.scalar.activation(
                        t[:], psq[:], AF.Relu, bias=qb_bias, scale=1.0
                    )
                    qk.append(t)
                # v_n: [v,k] natural layout, relu (+bias via rank-1 matmul)
                vn = []
                for vt in range(NVT):
                    t = sb.tile([128, KD], F32R, name=f"vn{b}_{vt}", tag="vn", bufs=3)
                    for ks in range(KD // NQ):
                        psv = ps.tile(
                            [128, NQ], F32, name=f"psv{b}_{vt}_{ks}", tag="ps_proj",
                            bufs=2,
                        )
                        nc.tensor.matmul(
                            psv[:],
                            vt_in[:, vt * 128 : (vt + 1) * 128],
                            w_vwt[:, ks * NQ : (ks + 1) * NQ],
                            start=True,
                            stop=vb_zero,
                        )
                        if not vb_zero:
                            nc.tensor.matmul(
                                psv[:],
                                w_ones[:],
                                w_vbrow[:, ks * NQ : (ks + 1) * NQ],
                                start=False,
                                stop=True,
                            )
                        nc.scalar.activation(
                            t[:, ks * NQ : (ks + 1) * NQ], psv[:], AF.Relu
                        )
                    vn.append(t)
                # v_kT: [k,v] k-major, relu(+bias)
                vk = []
                for kc in range(KC):
                    psk = ps.tile(
                        [128, NV], F32, name=f"psk{b}_{kc}", tag="ps_proj", bufs=2
                    )
                    nc.tensor.matmul(
                        psk[:],
                        w_vwt[:, kc * 128 : (kc + 1) * 128],
                        vt_in[:],
                        start=True,
                        stop=True,
                    )
                    t = sb.tile([128, NV], F32R, name=f"vk{b}_{kc}", tag="vk", bufs=12)
                    vb_bias = 0.0 if vb_zero else w_vbk[:, kc : kc + 1]
                    nc.scalar.activation(
                        t[:], psk[:], AF.Relu, bias=vb_bias, scale=1.0
                    )
                    vk.append(t)
                projs[b] = (qk, vn, vk)

            def emit_heads(b, pending=None):
                qk, vn, vk = projs[b]
                d = loads[b]
                nm = [
                    sb.tile([128, H_OUT], F32, name=f"nm{b}_{vt}", tag="nm", bufs=4)
                    for vt in range(NVT)
                ]
                rr = [
                    sb.tile([128, H_OUT], F32, name=f"rr{b}_{vt}", tag="rr", bufs=4)
                    for vt in range(NVT)
                ]
                att_sum = [
                    sb.tile([128, NQ], F32R, name=f"asum{b}_{vt}", tag="asum", bufs=4)
                    for vt in range(NVT)
                ]
                zs = {}

                def emit_vh(h):
                    vhs = []
                    for kc in range(KC):
                        t = sb.tile(
                            [128, NV], F32R, name=f"vh{b}_{h}_{kc}", tag="vh", bufs=26
                        )
                        c = kc * H_OUT + h
                        nc.vector.tensor_scalar_mul(
                            t[:], vk[kc][:].bitcast(F32), w_hmt[:, c : c + 1]
                        )
                        vhs.append(t)
                    return vhs

                vh_next = emit_vh(0)
                for h in range(H_OUT):
                    vh = vh_next
                    if h + 1 < H_OUT:
                        vh_next = emit_vh(h + 1)
                    psa = [
                        ps.tile(
                            [128, NQ], F32, name=f"psa{b}_{h}_{vt}", tag="ps_att",
                            bufs=4,
                        )
                        for vt in range(NVT)
                    ]
                    for kc in range(KC):
                        for vt in range(NVT):
                            nc.tensor.matmul(
                                psa[vt][:],
                                vh[kc][:, vt * 128 : (vt + 1) * 128],
                                qk[kc][:],
                                start=(kc == 0),
                                stop=(kc == KC - 1),
                            )
                    for vt in range(NVT):
                        if do_softmax:
                            if use_vmask:
                                nc.vector.tensor_scalar_add(
                                    psa[vt][:], psa[vt][:], d["vm_c"][:, vt : vt + 1]
                                )
                            if use_qmask:
                                nc.vector.tensor_add(
                                    psa[vt][:], psa[vt][:], d["qm_f"][:]
                                )
                            nc.vector.reduce_max(
                                nm[vt][:, h : h + 1], psa[vt][:], axis=AX.X,
                                negate=True,
                            )
                            z = sb.tile(
                                [128, NQ], BF16, name=f"z{b}_{h}_{vt}", tag="z",
                                bufs=18,
                            )
                            nc.scalar.activation(
                                z[:],
                                psa[vt][:],
                                AF.Exp,
                                bias=nm[vt][:, h : h + 1],
                                scale=1.0,
                                accum_out=rr[vt][:, h : h + 1],
                            )
                            zs[(h, vt)] = z
                        else:
                            ao = sb.tile(
                                [128, NQ], F32, name=f"ao{b}_{h}_{vt}", tag="ao",
                                bufs=3,
                            )
                            nc.vector.tensor_scalar_add(
                                ao[:], psa[vt][:], hb_cols[:, h : h + 1]
                            )
                            nc.sync.dma_start(
                                out=att_o[b, h, vt * 128 : (vt + 1) * 128, :],
                                in_=ao[:],
                            )
                            if h == 0:
                                nc.vector.tensor_copy(att_sum[vt][:], ao[:])
                            else:
                                nc.vector.tensor_add(
                                    att_sum[vt][:],
                                    ao[:],
                                    att_sum[vt][:].bitcast(F32),
                                )
                    # interleave deferred normalize/DMA work from the previous batch
                    if pending:
                        for _ in range(2):
                            if pending:
                                pending.pop(0)()
                while pending:
                    pending.pop(0)()
                return nm, rr, att_sum, zs

            def emit_stats_accum(b, nm, rr, att_sum, zs):
                # global per-(b,h) softmax constants from per-row stats
                m = []
                for vt in range(NVT):
                    t = sb.tile([128, H_OUT], F32, name=f"m{b}_{vt}", tag="mm", bufs=4)
                    nc.vector.tensor_scalar_mul(t[:], nm[vt][:], -1.0)
                    m.append(t)
                mg = sb.tile([128, H_OUT], F32, name=f"mg{b}", tag="mg", bufs=4)
                nc.vector.tensor_max(mg[:], m[0][:], m[1][:])
                Mt = sb.tile([128, H_OUT], F32, name=f"Mt{b}", tag="Mt", bufs=4)
                nc.gpsimd.partition_all_reduce(
                    Mt[:], mg[:], channels=128, reduce_op=bass_isa.ReduceOp.max
                )
                ws = sb.tile([128, H_OUT], F32, name=f"ws{b}", tag="wsu", bufs=4)
                ee = []
                w0 = None
                for vt in range(NVT):
                    dd = sb.tile([128, H_OUT], F32, name=f"d{b}_{vt}", tag="dd", bufs=4)
                    nc.vector.tensor_sub(dd[:], m[vt][:], Mt[:])
                    e = sb.tile([128, H_OUT], F32, name=f"e{b}_{vt}", tag="ee", bufs=4)
                    nc.scalar.activation(e[:], dd[:], AF.Exp)
                    ee.append(e)
                    w = sb.tile([128, H_OUT], F32, name=f"wv{b}_{vt}", tag="wv", bufs=4)
                    nc.vector.tensor_mul(w[:], rr[vt][:], e[:])
                    if vt == 0:
                        w0 = w
                    else:
                        nc.vector.tensor_add(ws[:], w0[:], w[:])
                St = sb.tile([128, H_OUT], F32, name=f"St{b}", tag="Stt", bufs=4)
                nc.gpsimd.partition_all_reduce(
                    St[:], ws[:], channels=128, reduce_op=bass_isa.ReduceOp.add
                )
                rS = sb.tile([128, H_OUT], F32, name=f"rS{b}", tag="rSS", bufs=4)
                nc.vector.reciprocal(rS[:], St[:])
                cf = []
                for vt in range(NVT):
                    t = sb.tile(
                        [128, H_OUT], F32, name=f"cf{b}_{vt}", tag="cff", bufs=4
                    )
                    nc.vector.tensor_mul(t[:], ee[vt][:], rS[:])
                    cf.append(t)
                # accumulate att_sum straight from z so fusion can start ASAP
                for h in range(H_OUT):
                    for vt in range(NVT):
                        z = zs[(h, vt)]
                        if h == 0:
                            nc.vector.tensor_scalar(
                                att_sum[vt][:],
                                z[:],
                                cf[vt][:, h : h + 1],
                                float(H_OUT) * 1e-10,
                                op0=ALU.mult,
                                op1=ALU.add,
                            )
                        else:
                            nc.vector.scalar_tensor_tensor(
                                att_sum[vt][:],
                                z[:],
                                cf[vt][:, h : h + 1],
                                att_sum[vt][:].bitcast(F32),
                                op0=ALU.mult,
                                op1=ALU.add,
                            )
                # deferred per-head normalize (ACT) + DMA closures
                pending = []
                for h in range(H_OUT):
                    for vt in range(NVT):
                        def mk(h=h, vt=vt):
                            z = zs[(h, vt)]
                            ao = sb.tile(
                                [128, NQ], F32, name=f"ao{b}_{h}_{vt}", tag="ao",
                                bufs=3,
                            )
                            nc.scalar.activation(
                                ao[:],
                                z[:],
                                AF.Identity,
                                bias=w_eps[:],
                                scale=cf[vt][:, h : h + 1],
                            )
                            nc.sync.dma_start(
                                out=att_o[b, h, vt * 128 : (vt + 1) * 128, :],
                                in_=ao[:],
                            )
                        pending.append(mk)
                return pending

            def emit_fusion(b, att_sum):
                qk, vn, vk = projs[b]
                fu = sb.tile([128, KC], F32, name=f"fu{b}", tag="fu", bufs=2)
                for kc in range(KC):
                    psu = ps.tile(
                        [128, NQ], F32, name=f"psu{b}_{kc}", tag="ps_fus", bufs=2
                    )
                    for vt in range(NVT):
                        nc.tensor.matmul(
                            psu[:],
                            vn[vt][:, kc * 128 : (kc + 1) * 128],
                            att_sum[vt][:],
                            start=(vt == 0),
                            stop=(vt == NVT - 1),
                        )
                    fscr = sb.tile(
                        [128, NQ], F32, name=f"fscr{b}_{kc}", tag="fscr", bufs=2
                    )
                    nc.vector.scalar_tensor_tensor(
                        fscr[:],
                        psu[:],
                        1.0,
                        qk[kc][:].bitcast(F32),
                        op0=ALU.mult,
                        op1=ALU.mult,
                        accum_out=fu[:, kc : kc + 1],
                    )
                t4 = sb.tile([128, JC], F32, name=f"t4{b}", tag="st4", bufs=2)
                nc.vector.tensor_add(t4[:], fu[:, 0:JC], fu[:, JC : 2 * JC])
                nc.vector.tensor_add(P[:, :, b], t4[:], fu[:, 2 * JC : 3 * JC])
                del projs[b]

            # -------- software-pipelined emission --------
            pending = None
            emit_loads(0)
            for b in range(BL):
                if b == 0:
                    emit_proj(0)
                if b + 1 < BL:
                    emit_loads(b + 1)
                nm, rr, att_sum, zs = emit_heads(b, pending)
                if b + 1 < BL:
                    emit_proj(b + 1)
                if do_softmax:
                    pending = emit_stats_accum(b, nm, rr, att_sum, zs)
                emit_fusion(b, att_sum)
            while pending:
                pending.pop(0)()

            # ---- BatchNorm over the full batch via AllReduce ----
            S12 = sb.tile([128, 2 * JC], F32)
            nc.vector.reduce_sum(S12[:, 0:JC], P[:], axis=AX.X)
            Psq = sb.tile([128, JC, BL], F32)
            nc.vector.tensor_mul(Psq[:], P[:], P[:])
            nc.vector.reduce_sum(S12[:, JC : 2 * JC], Psq[:], axis=AX.X)

            ccin = dram.tile([128, 2 * JC], F32)
            ccout = nc.dram_tensor("ccout", [128, 2 * JC], F32, addr_space="Shared")
            nc.gpsimd.dma_start(out=ccin[:], in_=S12[:])
            nc.gpsimd.collective_compute(
                "AllReduce",
                ALU.add,
                replica_groups=[list(range(N_CORES))],
                ins=[ccin[:]],
                outs=[ccout[:]],
            )
            Rt = sb.tile([128, 2 * JC], F32)
            nc.gpsimd.dma_start(out=Rt[:], in_=ccout[:])

            mn = sb.tile([128, JC], F32)
            nc.vector.tensor_scalar_mul(mn[:], Rt[:, 0:JC], 1.0 / B)
            e2 = sb.tile([128, JC], F32)
            nc.vector.tensor_scalar_mul(e2[:], Rt[:, JC : 2 * JC], 1.0 / B)
            vr = sb.tile([128, JC], F32)
            nc.vector.tensor_mul(vr[:], mn[:], mn[:])
            nc.vector.tensor_sub(vr[:], e2[:], vr[:])
            nc.vector.tensor_scalar_add(vr[:], vr[:], 1e-5)
            sd = sb.tile([128, JC], F32)
            nc.scalar.activation(sd[:], vr[:], AF.Sqrt)
            rstd = sb.tile([128, JC], F32)
            nc.vector.reciprocal(rstd[:], sd[:])
            sc = sb.tile([128, JC], F32)
            nc.vector.tensor_mul(sc[:], w_bng[:], rstd[:])
            for b in range(BL):
                t1 = sb.tile([128, JC], F32, name=f"lg1{b}", tag="st4", bufs=2)
                nc.vector.tensor_sub(t1[:], P[:, :, b], mn[:])
                nc.vector.tensor_mul(t1[:], t1[:], sc[:])
                nc.vector.tensor_add(LG[:, :, b], t1[:], w_bnb[:])
            for b in range(BL):
                nc.gpsimd.dma_start(
                    out=log_o[b].rearrange("(jc p) -> p jc", p=128),
                    in_=LG[:, :, b],
                )

    nc.compile()
    return nc


_CACHE = {}
_LAST_IN_MAPS = None


def _get_nc(key):
    if key not in _CACHE:
        _CACHE[key] = _build(*key)
    return _CACHE[key]


def kernel(
    v, q, v_mask, q_mask, softmax, v_W, v_b, q_W, q_b, h_mat, h_bias,
    bn_gamma, bn_beta,
):
    v = np.asarray(v, np.float32)
    q = np.asarray(q, np.float32)
    v_mask = np.asarray(v_mask)
    q_mask = np.asarray(q_mask)
    do_softmax = bool(np.asarray(softmax).item())
    use_vmask = do_softmax and not bool(np.all(v_mask != 0))
    use_qmask = do_softmax and not bool(np.all(q_mask != 0))
    vb_zero = bool(np.all(np.asarray(v_b) == 0))
    qb_zero = bool(np.all(np.asarray(q_b) == 0))

    kp = _KPERM
    v_Wp = np.asarray(v_W, np.float32)[kp]
    q_Wp = np.asarray(q_W, np.float32)[kp]
    v_bp = np.asarray(v_b, np.float32)[kp]
    q_bp = np.asarray(q_b, np.float32)[kp]
    h_mp = np.asarray(h_mat, np.float32)[:, kp]

    nc = _get_nc((do_softmax, use_vmask, use_qmask, vb_zero, qb_zero))

    # host-side shared (replicated) tensors
    qWT = _tf32(np.ascontiguousarray(q_Wp.T))  # [768, 1536]
    vWT = _tf32(np.ascontiguousarray(v_Wp.T))  # [128, 1536]
    onesr = np.ones((1, 128), np.float32)
    # hmt[p, kc*8+h] = h_mp[h, kc*128+p]
    hmt = np.ascontiguousarray(
        h_mp.reshape(H_OUT, KC, 128).transpose(2, 1, 0).reshape(128, KC * H_OUT)
    ).astype(np.float32)
    bng = np.ascontiguousarray(
        np.asarray(bn_gamma, np.float32).reshape(JC, 128).T
    )
    bnb = np.ascontiguousarray(
        np.asarray(bn_beta, np.float32).reshape(JC, 128).T
    )
    hbrow = _tf32(np.asarray(h_bias, np.float32)[None, :])

    in_maps = []
    for c in range(N_CORES):
        sl = slice(c * BL, (c + 1) * BL)
        m = {
            "vT": _tf32(np.ascontiguousarray(v[sl].transpose(0, 2, 1))),
            "qT": _tf32(np.ascontiguousarray(q[sl].transpose(0, 2, 1))),
            "qWT": qWT,
            "vWT": vWT,
            "onesr": onesr,
            "hmt": hmt,
            "bng": bng,
            "bnb": bnb,
            "hbrow": hbrow,
        }
        if not vb_zero:
            m["vbrow"] = _tf32(v_bp[None, :])
            m["vbk"] = np.ascontiguousarray(v_bp.reshape(KC, 128).T).astype(
                np.float32
            )
        if not qb_zero:
            m["qbk"] = np.ascontiguousarray(q_bp.reshape(KC, 128).T).astype(
                np.float32
            )
        if use_vmask:
            m["vmadd"] = ((v_mask[sl] != 0).astype(np.float32) - 1.0) * 1e9
        if use_qmask:
            m["qmadd"] = ((q_mask[sl] != 0).astype(np.float32) - 1.0) * 1e9
        in_maps.append(m)

    global _LAST_IN_MAPS
    _LAST_IN_MAPS = in_maps
    res = run_bass_kernel_spmd(nc, in_maps, list(range(N_CORES)))

    att = np.concatenate([res.results[c]["att"] for c in range(N_CORES)], axis=0)
    logits = np.concatenate(
        [res.results[c]["logits"] for c in range(N_CORES)], axis=0
    )
    return logits, att


# revision 21
# speedup vs baseline: 1.0711x; 1.0489x over previous
"""BANLayer (bilinear attention network) Trainium2 kernel, 8-core data-parallel.

Strategy (per core, 4 of the 32 batches):
  - All big matmuls in fp32r (TF32: exact on host-pre-rounded inputs, bf16-rate).
  - k-dimension globally permuted (host side) so AvgPool groups of 3 become
    column-tile adds: new k position p maps to original k = 3*(p%512) + p//512.
  - q_kT [k,q] and v_kT [k,v] (k-major) feed the per-head bilinear matmul;
    v_n [v,k] feeds the fusion einsum contraction over v; the fusion
    reduction over q happens elementwise in k-major layout (no transposes).
  - Softmax over the flat (v,q) grid per (b,h): per-row shift + exp (ACT,
    fused row sums), global max/sum via gpsimd partition_all_reduce on
    [128,8] stat tiles (8 heads batched), then per-row rescale on ACT.
  - Software pipelining: projections of batch b+1 are emitted between the
    head loop and the softmax-stats tail of batch b so the PE never idles.
  - BatchNorm batch stats via an 8-core AllReduce of [128,8] partial sums.

kernel(**inputs) takes the FULL unsharded inputs and returns (logits, att)
exactly like the reference module.
"""

import os
import sys

import numpy as np

try:
    import concourse.bass as bass
except ImportError:  # fresh grading dir: fall back to the repo checkout
    sys.path.insert(0, "/opt/trn_rl_repo")
    import concourse.bass as bass

import concourse.bacc as bacc
import concourse.bass_isa as bass_isa
import concourse.mybir as mybir
import concourse.tile as tile
from concourse.bass_utils import run_bass_kernel_spmd

F32 = mybir.dt.float32
F32R = mybir.dt.float32r
BF16 = mybir.dt.bfloat16
AX = mybir.AxisListType
AF = mybir.ActivationFunctionType
ALU = mybir.AluOpType

B, NV, NQ = 32, 256, 512
V_DIM, Q_DIM, H_DIM, H_OUT, K = 128, 768, 512, 8, 3
KD = H_DIM * K  # 1536
N_CORES = 8
BL = int(os.environ.get("BAN_BL", B // N_CORES))  # local batches per core
KC = KD // 128  # 12 k-chunks
DC = Q_DIM // 128  # 6 d-chunks for q
NVT = NV // 128  # 2 v partition tiles
JC = H_DIM // 128  # 4 output feature chunks


def _tf32(x):
    b = np.ascontiguousarray(x, np.float32).view(np.uint32)
    b = ((b.astype(np.uint64) + 0x1000) & 0xFFFFE000).astype(np.uint32)
    return b.view(np.float32)


# new k position p holds original k index 3*(p % 512) + p // 512
_KPERM = (3 * (np.arange(KD) % H_DIM) + np.arange(KD) // H_DIM).astype(np.int64)


def _build(do_softmax, use_vmask, use_qmask, vb_zero, qb_zero):
    nc = bacc.Bacc(
        "TRN2", target_bir_lowering=False, debug=False, num_devices=N_CORES
    )

    # ---- DRAM I/O ----
    vT = nc.dram_tensor("vT", [BL, 128, NV], F32R, kind="ExternalInput")
    qT = nc.dram_tensor("qT", [BL, Q_DIM, NQ], F32R, kind="ExternalInput")
    qWT = nc.dram_tensor("qWT", [Q_DIM, KD], F32R, kind="ExternalInput")
    vWT = nc.dram_tensor("vWT", [128, KD], F32R, kind="ExternalInput")
    onesr = nc.dram_tensor("onesr", [1, 128], F32R, kind="ExternalInput")
    hmt = nc.dram_tensor("hmt", [128, KC * H_OUT], F32, kind="ExternalInput")
    bng = nc.dram_tensor("bng", [128, JC], F32, kind="ExternalInput")
    bnb = nc.dram_tensor("bnb", [128, JC], F32, kind="ExternalInput")
    hbrow = nc.dram_tensor("hbrow", [1, H_OUT], F32R, kind="ExternalInput")
    if not vb_zero:
        vbrow = nc.dram_tensor("vbrow", [1, KD], F32R, kind="ExternalInput")
        vbk = nc.dram_tensor("vbk", [128, KC], F32, kind="ExternalInput")
    if not qb_zero:
        qbk = nc.dram_tensor("qbk", [128, KC], F32, kind="ExternalInput")
    if use_vmask:
        vmadd = nc.dram_tensor("vmadd", [BL, NV], F32, kind="ExternalInput")
    if use_qmask:
        qmadd = nc.dram_tensor("qmadd", [BL, NQ], F32, kind="ExternalInput")

    att_o = nc.dram_tensor("att", [BL, H_OUT, NV, NQ], F32, kind="ExternalOutput")
    log_o = nc.dram_tensor("logits", [BL, H_DIM], F32, kind="ExternalOutput")

    with tile.TileContext(nc) as tc:
        with (
            tc.tile_pool(name="wts", bufs=1) as wts,
            tc.tile_pool(name="io", bufs=1) as io,
            tc.tile_pool(name="sb", bufs=1) as sb,
            tc.tile_pool(name="ps", bufs=1, space="PSUM") as ps,
            tc.tile_pool(name="dram", bufs=1, space="DRAM") as dram,
        ):
            # ---- load constants ----
            w_qwt = []
            for dc in range(DC):
                t = wts.tile([128, KD], F32R, name=f"w_qwt{dc}", tag="w_qwt", bufs=DC)
                nc.gpsimd.dma_start(out=t[:], in_=qWT[dc * 128 : (dc + 1) * 128, :])
                w_qwt.append(t)
            w_vwt = wts.tile([128, KD], F32R)
            nc.gpsimd.dma_start(out=w_vwt[:], in_=vWT[:])
            w_ones = wts.tile([1, 128], F32R)
            nc.sync.dma_start(out=w_ones[:], in_=onesr[:])
            w_hmt = wts.tile([128, KC * H_OUT], F32)
            nc.sync.dma_start(out=w_hmt[:], in_=hmt[:])
            w_bng = wts.tile([128, JC], F32)
            nc.sync.dma_start(out=w_bng[:], in_=bng[:])
            w_bnb = wts.tile([128, JC], F32)
            nc.sync.dma_start(out=w_bnb[:], in_=bnb[:])
            w_hbrow = wts.tile([1, H_OUT], F32R)
            nc.sync.dma_start(out=w_hbrow[:], in_=hbrow[:])
            if not vb_zero:
                w_vbrow = wts.tile([1, KD], F32R)
                nc.sync.dma_start(out=w_vbrow[:], in_=vbrow[:])
                w_vbk = wts.tile([128, KC], F32)
                nc.sync.dma_start(out=w_vbk[:], in_=vbk[:])
            if not qb_zero:
                w_qbk = wts.tile([128, KC], F32)
                nc.sync.dma_start(out=w_qbk[:], in_=qbk[:])

            w_eps = wts.tile([128, 1], F32)
            nc.vector.memset(w_eps[:], 1e-10)

            hb_cols = None
            if not do_softmax:
                ps_hb = ps.tile([128, H_OUT], F32, tag="ps_proj", bufs=2)
                nc.tensor.matmul(
                    ps_hb[:], w_ones[:], w_hbrow[:], start=True, stop=True
                )
                hb_cols = sb.tile([128, H_OUT], F32)
                nc.vector.tensor_copy(hb_cols[:], ps_hb[:])

            # P[p, jc, b] accumulates pooled features
            P = sb.tile([128, JC, BL], F32)
            LG = sb.tile([128, JC, BL], F32)

            # -------- emission helpers (each emits IR; Tile schedules) ------
            loads = {}

            def emit_loads(b):
                d = {}
                t = io.tile([128, NV], F32R, name=f"vt_in{b}", tag="vt", bufs=2)
                nc.sync.dma_start(out=t[:], in_=vT[b])
                d["vt_in"] = t
                d["qt_in"] = []
                for dc in range(DC):
                    t = io.tile(
                        [128, NQ], F32R, name=f"qt_in{b}_{dc}", tag="qt", bufs=10
                    )
                    nc.sync.dma_start(
                        out=t[:], in_=qT[b, dc * 128 : (dc + 1) * 128, :]
                    )
                    d["qt_in"].append(t)
                if use_vmask:
                    t = io.tile([128, NVT], F32, name=f"vm{b}", tag="vm", bufs=2)
                    nc.sync.dma_start(
                        out=t[:], in_=vmadd[b].rearrange("(vt p) -> p vt", p=128)
                    )
                    d["vm_c"] = t
                if use_qmask:
                    qm_r = io.tile([1, NQ], F32, name=f"qmr{b}", tag="qmr", bufs=2)
                    nc.sync.dma_start(out=qm_r[:], in_=qmadd[b : b + 1, :])
                    qm_f = io.tile([128, NQ], F32, name=f"qmf{b}", tag="qmf", bufs=2)
                    nc.gpsimd.partition_broadcast(qm_f[:], qm_r[:], channels=128)
                    d["qm_f"] = qm_f
                loads[b] = d

            projs = {}

            def emit_proj(b):
                d = loads[b]
                vt_in, qt_in = d["vt_in"], d["qt_in"]
                # q_kT: [k,q] k-major, relu(+bias)
                qk = []
                for kc in range(KC):
                    psq = ps.tile(
                        [128, NQ], F32, name=f"psq{b}_{kc}", tag="ps_proj", bufs=2
                    )
                    for dc in range(DC):
                        nc.tensor.matmul(
                            psq[:],
                            w_qwt[dc][:, kc * 128 : (kc + 1) * 128],
                            qt_in[dc][:],
                            start=(dc == 0),
                            stop=(dc == DC - 1),
                        )
                    t = sb.tile([128, NQ], F32R, name=f"qk{b}_{kc}", tag="qk", bufs=24)
                    qb_bias = 0.0 if qb_zero else w_qbk[:, kc : kc + 1]
                    nc.scalar.activation(
                        t[:], psq[:], AF.Relu, bias=qb_bias, scale=1.0
                    )
                    qk.append(t)
                # v_n: [v,k] natural layout, relu (+bias via rank-1 matmul)
                vn = []
                for vt in range(NVT):
                    t = sb.tile([128, KD], F32R, name=f"vn{b}_{vt}", tag="vn", bufs=3)
                    for ks in range(KD // NQ):
                        psv = ps.tile(
                            [128, NQ], F32, name=f"psv{b}_{vt}_{ks}", tag="ps_proj",
                            bufs=2,
                        )
                        nc.tensor.matmul(
                            psv[:],
                            vt_in[:, vt * 128 : (vt + 1) * 128],
                            w_vwt[:, ks * NQ : (ks + 1) * NQ],
                            start=True,
                            stop=vb_zero,
                        )
                        if not vb_zero:
                            nc.tensor.matmul(
                                psv[:],
                                w_ones[:],
                                w_vbrow[:, ks * NQ : (ks + 1) * NQ],
                                start=False,
                                stop=True,
                            )
                        nc.scalar.activation(
                            t[:, ks * NQ : (ks + 1) * NQ], psv[:], AF.Relu
                        )
                    vn.append(t)
                # v_kT: [k,v] k-major, relu(+bias)
                vk = []
                for kc in range(KC):
                    psk = ps.tile(
                        [128, NV], F32, name=f"psk{b}_{kc}", tag="ps_proj", bufs=2
                    )
                    nc.tensor.matmul(
                        psk[:],
                        w_vwt[:, kc * 128 : (kc + 1) * 128],
                        vt_in[:],
                        start=True,
                        stop=True,
                    )
                    t = sb.tile([128, NV], F32R, name=f"vk{b}_{kc}", tag="vk", bufs=12)
                    vb_bias = 0.0 if vb_zero else w_vbk[:, kc : kc + 1]
                    nc.scalar.activation(
                        t[:], psk[:], AF.Relu, bias=vb_bias, scale=1.0
                    )
                    vk.append(t)
                projs[b] = (qk, vn, vk)

            def emit_heads(b, pending=None):
                qk, vn, vk = projs[b]
                d = loads[b]
                nm = [
                    sb.tile([128, H_OUT], F32, name=f"nm{b}_{vt}", tag="nm", bufs=4)
                    for vt in range(NVT)
                ]
                rr = [
                    sb.tile([128, H_OUT], F32, name=f"rr{b}_{vt}", tag="rr", bufs=4)
                    for vt in range(NVT)
                ]
                att_sum = [
                    sb.tile([128, NQ], F32R, name=f"asum{b}_{vt}", tag="asum", bufs=4)
                    for vt in range(NVT)
                ]
                zs = {}

                def emit_vh(h):
                    vhs = []
                    for kc in range(KC):
                        t = sb.tile(
                            [128, NV], F32R, name=f"vh{b}_{h}_{kc}", tag="vh", bufs=26
                        )
                        c = kc * H_OUT + h
                        nc.vector.tensor_scalar_mul(
                            t[:], vk[kc][:].bitcast(F32), w_hmt[:, c : c + 1]
                        )
                        vhs.append(t)
                    return vhs

                vh_next = emit_vh(0)
                for h in range(H_OUT):
                    vh = vh_next
                    if h + 1 < H_OUT:
                        vh_next = emit_vh(h + 1)
                    psa = [
                        ps.tile(
                            [128, NQ], F32, name=f"psa{b}_{h}_{vt}", tag="ps_att",
                            bufs=4,
                        )
                        for vt in range(NVT)
                    ]
                    for kc in range(KC):
                        for vt in range(NVT):
                            nc.tensor.matmul(
                                psa[vt][:],
                                vh[kc][:, vt * 128 : (vt + 1) * 128],
                                qk[kc][:],
                                start=(kc == 0),
                                stop=(kc == KC - 1),
                            )
                    for vt in range(NVT):
                        if do_softmax:
                            if use_vmask:
                                nc.vector.tensor_scalar_add(
                                    psa[vt][:], psa[vt][:], d["vm_c"][:, vt : vt + 1]
                                )
                            if use_qmask:
                                nc.vector.tensor_add(
                                    psa[vt][:], psa[vt][:], d["qm_f"][:]
                                )
                            nc.vector.reduce_max(
                                nm[vt][:, h : h + 1], psa[vt][:], axis=AX.X,
                                negate=True,
                            )
                            z = sb.tile(
                                [128, NQ], BF16, name=f"z{b}_{h}_{vt}", tag="z",
                                bufs=18,
                            )
                            nc.scalar.activation(
                                z[:],
                                psa[vt][:],
                                AF.Exp,
                                bias=nm[vt][:, h : h + 1],
                                scale=1.0,
                                accum_out=rr[vt][:, h : h + 1],
                            )
                            zs[(h, vt)] = z
                        else:
                            ao = sb.tile(
                                [128, NQ], F32, name=f"ao{b}_{h}_{vt}", tag="ao",
                                bufs=3,
                            )
                            nc.vector.tensor_scalar_add(
                                ao[:], psa[vt][:], hb_cols[:, h : h + 1]
                            )
                            nc.sync.dma_start(
                                out=att_o[b, h, vt * 128 : (vt + 1) * 128, :],
                                in_=ao[:],
                            )
                            if h == 0:
                                nc.vector.tensor_copy(att_sum[vt][:], ao[:])
                            else:
                                nc.vector.tensor_add(
                                    att_sum[vt][:],
                                    ao[:],
                                    att_sum[vt][:].bitcast(F32),
                                )
                    # interleave deferred normalize/DMA work from the previous batch
                    if pending:
                        for _ in range(2):
                            if pending:
                                pending.pop(0)()
                while pending:
                    pending.pop(0)()
                return nm, rr, att_sum, zs

            def emit_stats_accum(b, nm, rr, att_sum, zs):
                # global per-(b,h) softmax constants from per-row stats
                m = []
                for vt in range(NVT):
                    t = sb.tile([128, H_OUT], F32, name=f"m{b}_{vt}", tag="mm", bufs=4)
                    nc.vector.tensor_scalar_mul(t[:], nm[vt][:], -1.0)
                    m.append(t)
                mg = sb.tile([128, H_OUT], F32, name=f"mg{b}", tag="mg", bufs=4)
                nc.vector.tensor_max(mg[:], m[0][:], m[1][:])
                Mt = sb.tile([128, H_OUT], F32, name=f"Mt{b}", tag="Mt", bufs=4)
                nc.gpsimd.partition_all_reduce(
                    Mt[:], mg[:], channels=128, reduce_op=bass_isa.ReduceOp.max
                )
                ws = sb.tile([128, H_OUT], F32, name=f"ws{b}", tag="wsu", bufs=4)
                ee = []
                w0 = None
                for vt in range(NVT):
                    dd = sb.tile([128, H_OUT], F32, name=f"d{b}_{vt}", tag="dd", bufs=4)
                    nc.vector.tensor_sub(dd[:], m[vt][:], Mt[:])
                    e = sb.tile([128, H_OUT], F32, name=f"e{b}_{vt}", tag="ee", bufs=4)
                    nc.scalar.activation(e[:], dd[:], AF.Exp)
                    ee.append(e)
                    w = sb.tile([128, H_OUT], F32, name=f"wv{b}_{vt}", tag="wv", bufs=4)
                    nc.vector.tensor_mul(w[:], rr[vt][:], e[:])
                    if vt == 0:
                        w0 = w
                    else:
                        nc.vector.tensor_add(ws[:], w0[:], w[:])
                St = sb.tile([128, H_OUT], F32, name=f"St{b}", tag="Stt", bufs=4)
                nc.gpsimd.partition_all_reduce(
                    St[:], ws[:], channels=128, reduce_op=bass_isa.ReduceOp.add
                )
                rS = sb.tile([128, H_OUT], F32, name=f"rS{b}", tag="rSS", bufs=4)
                nc.vector.reciprocal(rS[:], St[:])
                cf = []
                for vt in range(NVT):
                    t = sb.tile(
                        [128, H_OUT], F32, name=f"cf{b}_{vt}", tag="cff", bufs=4
                    )
                    nc.vector.tensor_mul(t[:], ee[vt][:], rS[:])
                    cf.append(t)
                # accumulate att_sum straight from z so fusion can start ASAP
                for h in range(H_OUT):
                    for vt in range(NVT):
                        z = zs[(h, vt)]
                        if h == 0:
                            nc.vector.tensor_scalar(
                                att_sum[vt][:],
                                z[:],
                                cf[vt][:, h : h + 1],
                                float(H_OUT) * 1e-10,
                                op0=ALU.mult,
                                op1=ALU.add,
                            )
                        else:
                            nc.vector.scalar_tensor_tensor(
                                att_sum[vt][:],
                                z[:],
                                cf[vt][:, h : h + 1],
                                att_sum[vt][:].bitcast(F32),
                                op0=ALU.mult,
                                op1=ALU.add,
                            )
                # deferred per-head normalize (ACT) + DMA closures
                pending = []
                for h in range(H_OUT):
                    for vt in range(NVT):
                        def mk(h=h, vt=vt):
                            z = zs[(h, vt)]
                            ao = sb.tile(
                                [128, NQ], F32, name=f"ao{b}_{h}_{vt}", tag="ao",
                                bufs=3,
                            )
                            nc.vector.tensor_scalar(
                                ao[:],
                                z[:],
                                cf[vt][:, h : h + 1],
                                1e-10,
                                op0=ALU.mult,
                                op1=ALU.add,
                            )
                            nc.sync.dma_start(
                                out=att_o[b, h, vt * 128 : (vt + 1) * 128, :],
                                in_=ao[:],
                            )
                        pending.append(mk)
                return pending

            def emit_fusion(b, att_sum):
                qk, vn, vk = projs[b]
                fu = sb.tile([128, KC], F32, name=f"fu{b}", tag="fu", bufs=2)
                for kc in range(KC):
                    psu = ps.tile(
                        [128, NQ], F32, name=f"psu{b}_{kc}", tag="ps_fus", bufs=2
                    )
                    for vt in range(NVT):
                        nc.tensor.matmul(
                            psu[:],
                            vn[vt][:, kc * 128 : (kc + 1) * 128],
                            att_sum[vt][:],
                            start=(vt == 0),
                            stop=(vt == NVT - 1),
                        )
                    fscr = sb.tile(
                        [128, NQ], F32, name=f"fscr{b}_{kc}", tag="fscr", bufs=2
                    )
                    nc.vector.scalar_tensor_tensor(
                        fscr[:],
                        psu[:],
                        1.0,
                        qk[kc][:].bitcast(F32),
                        op0=ALU.mult,
                        op1=ALU.mult,
                        accum_out=fu[:, kc : kc + 1],
                    )
                t4 = sb.tile([128, JC], F32, name=f"t4{b}", tag="st4", bufs=2)
                nc.vector.tensor_add(t4[:], fu[:, 0:JC], fu[:, JC : 2 * JC])
                nc.vector.tensor_add(P[:, :, b], t4[:], fu[:, 2 * JC : 3 * JC])
                del projs[b]

            # -------- software-pipelined emission --------
            pending = None
            emit_loads(0)
            for b in range(BL):
                if b == 0:
                    emit_proj(0)
                if b + 1 < BL:
                    emit_loads(b + 1)
                nm, rr, att_sum, zs = emit_heads(b, pending)
                if b + 1 < BL:
                    emit_proj(b + 1)
                if do_softmax:
                    pending = emit_stats_accum(b, nm, rr, att_sum, zs)
                emit_fusion(b, att_sum)
            while pending:
                pending.pop(0)()

            # ---- BatchNorm over the full batch via AllReduce ----
            S12 = sb.tile([128, 2 * JC], F32)
            nc.vector.reduce_sum(S12[:, 0:JC], P[:], axis=AX.X)
            Psq = sb.tile([128, JC, BL], F32)
            nc.vector.tensor_mul(Psq[:], P[:], P[:])
            nc.vector.reduce_sum(S12[:, JC : 2 * JC], Psq[:], axis=AX.X)

            ccin = dram.tile([128, 2 * JC], F32)
            ccout = nc.dram_tensor("ccout", [128, 2 * JC], F32, addr_space="Shared")
            nc.gpsimd.dma_start(out=ccin[:], in_=S12[:])
            nc.gpsimd.collective_compute(
                "AllReduce",
                ALU.add,
                replica_groups=[list(range(N_CORES))],
                ins=[ccin[:]],
                outs=[ccout[:]],
            )
            Rt = sb.tile([128, 2 * JC], F32)
            nc.gpsimd.dma_start(out=Rt[:], in_=ccout[:])

            mn = sb.tile([128, JC], F32)
            nc.vector.tensor_scalar_mul(mn[:], Rt[:, 0:JC], 1.0 / B)
            e2 = sb.tile([128, JC], F32)
            nc.vector.tensor_scalar_mul(e2[:], Rt[:, JC : 2 * JC], 1.0 / B)
            vr = sb.tile([128, JC], F32)
            nc.vector.tensor_mul(vr[:], mn[:], mn[:])
            nc.vector.tensor_sub(vr[:], e2[:], vr[:])
            nc.vector.tensor_scalar_add(vr[:], vr[:], 1e-5)
            sd = sb.tile([128, JC], F32)
            nc.scalar.activation(sd[:], vr[:], AF.Sqrt)
            rstd = sb.tile([128, JC], F32)
            nc.vector.reciprocal(rstd[:], sd[:])
            sc = sb.tile([128, JC], F32)
            nc.vector.tensor_mul(sc[:], w_bng[:], rstd[:])
            for b in range(BL):
                t1 = sb.tile([128, JC], F32, name=f"lg1{b}", tag="st4", bufs=2)
                nc.vector.tensor_sub(t1[:], P[:, :, b], mn[:])
                nc.vector.tensor_mul(t1[:], t1[:], sc[:])
                nc.vector.tensor_add(LG[:, :, b], t1[:], w_bnb[:])
            for b in range(BL):
                nc.gpsimd.dma_start(
                    out=log_o[b].rearrange("(jc p) -> p jc", p=128),
                    in_=LG[:, :, b],
                )

    nc.compile()
    return nc


_CACHE = {}
_LAST_IN_MAPS = None


def _get_nc(key):
    if key not in _CACHE:
        _CACHE[key] = _build(*key)
    return _CACHE[key]


def kernel(
    v, q, v_mask, q_mask, softmax, v_W, v_b, q_W, q_b, h_mat, h_bias,
    bn_gamma, bn_beta,
):
    v = np.asarray(v, np.float32)
    q = np.asarray(q, np.float32)
    v_mask = np.asarray(v_mask)
    q_mask = np.asarray(q_mask)
    do_softmax = bool(np.asarray(softmax).item())
    use_vmask = do_softmax and not bool(np.all(v_mask != 0))
    use_qmask = do_softmax and not bool(np.all(q_mask != 0))
    vb_zero = bool(np.all(np.asarray(v_b) == 0))
    qb_zero = bool(np.all(np.asarray(q_b) == 0))

    kp = _KPERM
    v_Wp = np.asarray(v_W, np.float32)[kp]
    q_Wp = np.asarray(q_W, np.float32)[kp]
    v_bp = np.asarray(v_b, np.float32)[kp]
    q_bp = np.asarray(q_b, np.float32)[kp]
    h_mp = np.asarray(h_mat, np.float32)[:, kp]

    nc = _get_nc((do_softmax, use_vmask, use_qmask, vb_zero, qb_zero))

    # host-side shared (replicated) tensors
    qWT = _tf32(np.ascontiguousarray(q_Wp.T))  # [768, 1536]
    vWT = _tf32(np.ascontiguousarray(v_Wp.T))  # [128, 1536]
    onesr = np.ones((1, 128), np.float32)
    # hmt[p, kc*8+h] = h_mp[h, kc*128+p]
    hmt = np.ascontiguousarray(
        h_mp.reshape(H_OUT, KC, 128).transpose(2, 1, 0).reshape(128, KC * H_OUT)
    ).astype(np.float32)
    bng = np.ascontiguousarray(
        np.asarray(bn_gamma, np.float32).reshape(JC, 128).T
    )
    bnb = np.ascontiguousarray(
        np.asarray(bn_beta, np.float32).reshape(JC, 128).T
    )
    hbrow = _tf32(np.asarray(h_bias, np.float32)[None, :])

    in_maps = []
    for c in range(N_CORES):
        sl = slice(c * BL, (c + 1) * BL)
        m = {
            "vT": _tf32(np.ascontiguousarray(v[sl].transpose(0, 2, 1))),
            "qT": _tf32(np.ascontiguousarray(q[sl].transpose(0, 2, 1))),
            "qWT": qWT,
            "vWT": vWT,
            "onesr": onesr,
            "hmt": hmt,
            "bng": bng,
            "bnb": bnb,
            "hbrow": hbrow,
        }
        if not vb_zero:
            m["vbrow"] = _tf32(v_bp[None, :])
            m["vbk"] = np.ascontiguousarray(v_bp.reshape(KC, 128).T).astype(
                np.float32
            )
        if not qb_zero:
            m["qbk"] = np.ascontiguousarray(q_bp.reshape(KC, 128).T).astype(
                np.float32
            )
        if use_vmask:
            m["vmadd"] = ((v_mask[sl] != 0).astype(np.float32) - 1.0) * 1e9
        if use_qmask:
            m["qmadd"] = ((q_mask[sl] != 0).astype(np.float32) - 1.0) * 1e9
        in_maps.append(m)

    global _LAST_IN_MAPS
    _LAST_IN_MAPS = in_maps
    res = run_bass_kernel_spmd(nc, in_maps, list(range(N_CORES)))

    att = np.concatenate([res.results[c]["att"] for c in range(N_CORES)], axis=0)
    logits = np.concatenate(
        [res.results[c]["logits"] for c in range(N_CORES)], axis=0
    )
    return logits, att
